# revision 6
# baseline (speedup 1.0000x reference)
"""Trainium2 Bass kernel for nn_CustomMatrixMultiplication.

Computes out[b, m] = sum_{n,p} m1[b, n, m] * m2[b, p, n]
              = sum_n m1[b, n, m] * s[b, n],   s[b, n] = sum_p m2[b, p, n]

Sharding: pure data parallel over batch B=64 across 8 NeuronCores
(8 batches per core). Each core:
  stage 1: s = ones.T @ m2[b]   (PE matmul, f32r, partition-dim reduction)
  relayout: s [1,1024] -> sT [128,8] via tiny SBUF->SBUF scatter DMA
            (row p=8i+r of the contraction lives on partition i, so both
             stages use the same mod-8 row grouping and no transpose is
             needed anywhere)
  stage 2: out = sum_g sT[:,g].T @ m1tile[g]  (PE matmul, f32r)
f32r (tf32-like, ~1e-4 rel) keeps the PE at 1 cycle/row for 512-wide
moving operands; accumulation is fp32 in PSUM.
"""

from contextlib import ExitStack

import numpy as np

import concourse.bacc as bacc
import concourse.mybir as mybir
import concourse.tile as tile
from concourse.bass_utils import run_bass_kernel_spmd

dt = mybir.dt

B, N, M, P = 64, 1024, 1024, 1024
NCORES = 8
BL = B // NCORES  # batches per core
H = 512           # matmul free-dim tile (fp32 moving-operand max)
R = 8             # row groups of 128 (1024 contraction rows / 128 partitions)
R2 = R // 2       # row groups per 2MB load half

_cache = {}


def _build():
    nc = bacc.Bacc(None, target_bir_lowering=False)
    m1_d = nc.dram_tensor("matrix1", [BL, N, M], dt.float32r, kind="ExternalInput")
    m2_d = nc.dram_tensor("matrix2", [BL, P, N], dt.float32r, kind="ExternalInput")
    out_d = nc.dram_tensor("out", [BL, M], dt.float32, kind="ExternalOutput")

    with tile.TileContext(nc) as tc, ExitStack() as ctx:
        big = ctx.enter_context(tc.tile_pool(name="big", bufs=8))
        small = ctx.enter_context(tc.tile_pool(name="small", bufs=2))
        const = ctx.enter_context(tc.tile_pool(name="const", bufs=1))
        psum = ctx.enter_context(tc.tile_pool(name="psum", bufs=3, space="PSUM"))

        ones_f32 = const.tile([128, 1], dt.float32)
        nc.vector.memset(ones_f32[:], 1.0)
        ones = const.tile([128, 1], dt.float32r)
        nc.vector.tensor_copy(ones[:], ones_f32[:])
        o_all = const.tile([1, BL * M], dt.float32)

        m1ts = [None] * BL
        sTs = [None] * BL

        def stage2(b):
            # stage 2: out[m] = sum_g sum_i m1[8i+g, m] * s[8i+g]
            sT, (m1ta, m1tb) = sTs[b], m1ts[b]
            ps_o = psum.tile([1, M], dt.float32, tag="ps")
            for half, m1t in ((0, m1ta), (1, m1tb)):
                for h in range(M // H):
                    for g in range(half * R2, half * R2 + R2):
                        nc.tensor.matmul(
                            ps_o[0:1, H * h : H * (h + 1)],
                            sT[:, g : g + 1],
                            m1t[:, g - half * R2, H * h : H * (h + 1)],
                            start=(g == 0),
                            stop=(g == R - 1),
                        )
            nc.vector.tensor_copy(o_all[0:1, M * b : M * (b + 1)], ps_o[:])

        for b in range(BL):
            # load m2[b] in two 2MB halves: row 8i+r -> partition i,
            # free [r, n]; contiguous source (128 descriptors x 16KB each)
            m2_ap = m2_d[b].rearrange("(p r) n -> p r n", p=128)
            ps_s = psum.tile([1, N], dt.float32, tag="ps")
            for half in range(2):
                m2t = big.tile([128, R2, N], dt.float32r, tag="big")
                nc.sync.dma_start(m2t[:], m2_ap[:, half * R2 : half * R2 + R2, :])
                # stage 1: s[n] = sum_r sum_i m2[8i+r, n]
                for h in range(N // H):
                    for r in range(half * R2, half * R2 + R2):
                        nc.tensor.matmul(
                            ps_s[0:1, H * h : H * (h + 1)],
                            ones[:],
                            m2t[:, r - half * R2, H * h : H * (h + 1)],
                            start=(r == 0),
                            stop=(r == R - 1),
                        )
            s_b = small.tile([1, N], dt.float32r, tag="s")
            nc.vector.tensor_copy(s_b[:], ps_s[:])  # rounds to f32r

            # relayout: sT[i, g] = s[8i + g]
            sT = small.tile([128, R], dt.float32r, tag="sT")
            nc.scalar.dma_start(sT[:], s_b[:])
            sTs[b] = sT

            # load m1[b] with the same mod-8 row grouping, in halves
            m1_ap = m1_d[b].rearrange("(p r) m -> p r m", p=128)
            pair = []
            for half in range(2):
                m1t = big.tile([128, R2, M], dt.float32r, tag="big")
                nc.sync.dma_start(m1t[:], m1_ap[:, half * R2 : half * R2 + R2, :])
                pair.append(m1t)
            m1ts[b] = pair

            # software pipeline: emit stage 2 of the PREVIOUS batch here so
            # the PE never head-of-line blocks on batch b's scatter DMA
            if b >= 1:
                stage2(b - 1)
        stage2(BL - 1)

        nc.sync.dma_start(out_d[:, :], o_all[:])

    nc.finalize()
    return nc


def _get_nc():
    if "nc" not in _cache:
        _cache["nc"] = _build()
    return _cache["nc"]


def kernel(matrix1, matrix2, _run_kwargs=None):
    m1 = np.ascontiguousarray(np.asarray(matrix1, dtype=np.float32))
    m2 = np.ascontiguousarray(np.asarray(matrix2, dtype=np.float32))
    assert m1.shape == (B, N, M) and m2.shape == (B, P, N)

    nc = _get_nc()
    in_maps = [
        {
            "matrix1": m1[i * BL : (i + 1) * BL],
            "matrix2": m2[i * BL : (i + 1) * BL],
        }
        for i in range(NCORES)
    ]
    res = run_bass_kernel_spmd(
        nc, in_maps, core_ids=list(range(NCORES)), **(_run_kwargs or {})
    )
    out = np.concatenate([res.results[i]["out"] for i in range(NCORES)], axis=0)
    if _run_kwargs:
        _cache["last_results"] = res
    return out
